# revision 7
# baseline (speedup 1.0000x reference)
"""Trainium2 Bass kernel for MLP-with-SOM-cosine-similarity (retrieval_knn).

Reference computation per (b, k) pair:
  ctx, ent: [L=128, D=128] slices of context[b, k, 0/1]
  sim[l, m] = cos(ctx[l], ent[m]); idx[l] = argmax_m sim[l, m]
  x = [ctx_n | ent_n[idx]] -> 6x tanh(Linear(256,256)) -> dot W_out -> sum over l
Output: [B=64, K=64] f32.

Strategy: data-parallel over batch dim (8 cores x 8 batches = 512 pairs/core).

Engine budget (from trace analysis; per-core totals at 32 subgroups):
  - ACT (Scalar): tanh only, 24x [128,1024] instrs/subgroup = ~855us. Hard
    floor; nothing else may ride on ACT.
  - DVE: Newton rsqrt, ctxb/entb casts, PSUM->SBUF moves (cpt, gat), argmax
    (max reduce + is_equal), y6 row-sums. Target < ACT.
  - GpSimd: squares, norm reduce, normalize muls.
  - PE: fp32 transposes + fp32 sim (precision-mandatory: bf16 sim flips
    argmax -> 4.5e-2 rel err vs 2e-2 tol), bf16 MLP. Real streaming ~650us.
  - DMA xbar: one-hot transpose and ctx_nT bf16 chunk (16-bit only), freeing
    PE instrs and the DVE PSUM->SBUF copy they needed.
Pipeline (subgroup w): DMA+squares at w-3 | norms at w-2 | casts late at w-2
  | sim stage A (transposes+sim+argmax) mid-layers 1-4 of mlp(w-1), stage B
  (gather) mid-layers 2-5 | mlp at w | y6 row-sum at end of w | tiny wout
  matmuls mid-layer-1 of w+1. The mid-layer emission points keep the PE busy
  where it used to stall on the mm-ring waiting for ACT.
PSUM banks: tp(2) + sim(1) + scr/gat(1) + mlp(4, also hosts tiny wo) = 8.
"""

from contextlib import ExitStack

import numpy as np
import ml_dtypes

import concourse.bass as bass
import concourse.bacc as bacc
import concourse.tile as tile
from concourse import mybir
from concourse.alu_op_type import AluOpType
from concourse.bass_utils import run_bass_kernel_spmd
from concourse.masks import make_identity

BF16 = mybir.dt.bfloat16
F32 = mybir.dt.float32
AF = mybir.ActivationFunctionType

B, K, L, D = 64, 64, 128, 128
N_CORES = 8
PAIRS = (B // N_CORES) * K          # 512 pairs per core
N_HIDDEN = 6
SUB = 16                            # pairs per DMA subgroup
GRP = 4                             # pairs per PSUM group
UNROLL = 128                        # pairs per outer block

_cache = {}


def _build_bass():
    nc = bacc.Bacc("TRN2")

    ctx_dram = nc.dram_tensor("ctxpairs", [PAIRS, 2, L, D], F32, kind="ExternalInput")
    wt_dram = nc.dram_tensor("wt", [128, N_HIDDEN * 2 * 2 * 128], BF16, kind="ExternalInput")
    wout_dram = nc.dram_tensor("wout", [128, 2], BF16, kind="ExternalInput")
    bias_dram = nc.dram_tensor("bias", [128, N_HIDDEN * 2], F32, kind="ExternalInput")
    bout_dram = nc.dram_tensor("bout", [1, 1], F32, kind="ExternalInput")
    out_dram = nc.dram_tensor("out", [1, PAIRS], F32, kind="ExternalOutput")

    with ExitStack() as ctx:
        tc = ctx.enter_context(tile.TileContext(nc))
        const = ctx.enter_context(tc.tile_pool(name="const", bufs=1))
        raw_pool = ctx.enter_context(tc.tile_pool(name="raw", bufs=3))
        sq_pool = ctx.enter_context(tc.tile_pool(name="sq", bufs=2))
        norm_pool = ctx.enter_context(tc.tile_pool(name="norm", bufs=2))
        tiny_pool = ctx.enter_context(tc.tile_pool(name="tiny", bufs=4))
        pre_pool = ctx.enter_context(tc.tile_pool(name="pre", bufs=4))
        x_pool = ctx.enter_context(tc.tile_pool(name="xsb", bufs=4))
        y_pool = ctx.enter_context(tc.tile_pool(name="ysb", bufs=4))
        ybar_pool = ctx.enter_context(tc.tile_pool(name="ybar", bufs=4))
        res_pool = ctx.enter_context(tc.tile_pool(name="res", bufs=2))
        # PSUM: 8 banks total = tp(2) + sim(1) + scr(1) + mlp(4, also hosts wo)
        ps_tp = ctx.enter_context(tc.tile_pool(name="pstp", bufs=2, space="PSUM"))
        ps_sim = ctx.enter_context(tc.tile_pool(name="pssim", bufs=1, space="PSUM"))
        ps_scr = ctx.enter_context(tc.tile_pool(name="psscr", bufs=1, space="PSUM"))
        ps_mlp = ctx.enter_context(tc.tile_pool(name="psmlp", bufs=2, space="PSUM"))

        wt_sb = const.tile([128, N_HIDDEN, 2, 2, 128], BF16)
        nc.sync.dma_start(out=wt_sb, in_=wt_dram.rearrange("a (i kc mc b) -> a i kc mc b", i=N_HIDDEN, kc=2, mc=2))
        wout_sb = const.tile([128, 2], BF16)
        nc.sync.dma_start(out=wout_sb, in_=wout_dram[:, :])
        bias_sb = const.tile([128, N_HIDDEN * 2], F32)
        nc.sync.dma_start(out=bias_sb, in_=bias_dram[:, :])
        bout_sb = const.tile([1, 1], F32)
        nc.sync.dma_start(out=bout_sb, in_=bout_dram[:, :])
        ident = const.tile([128, 128], F32)
        make_identity(nc, ident)
        bout128 = const.tile([1, 1], F32)
        nc.vector.tensor_scalar(out=bout128, in0=bout_sb, scalar1=float(L), scalar2=0.0,
                                op0=AluOpType.mult, op1=AluOpType.add)

        n_blk = UNROLL // SUB           # subgroups per output block
        n_sub_total = PAIRS // SUB
        HS = SUB // 2

        def dma_sq_stage(s):
            """DMA subgroup s + squares on gpsimd. Emitted 3 subgroups ahead."""
            raw = raw_pool.tile([128, SUB, 2, 128], F32, tag="raw")
            nc.sync.dma_start(
                out=raw,
                in_=ctx_dram[s * SUB : s * SUB + SUB].rearrange("p c l d -> l p c d"),
            )
            sq = sq_pool.tile([128, SUB, 2, 128], F32, tag="sq")
            for hh in range(2):
                nc.gpsimd.tensor_mul(sq[:, hh * HS : hh * HS + HS],
                                     raw[:, hh * HS : hh * HS + HS],
                                     raw[:, hh * HS : hh * HS + HS])
            return raw, sq

        def norm_early(rawsq):
            """Norm reduce (gpsimd) + Newton rsqrt (DVE) + normalize (gpsimd).
            Emitted 2 subgroups ahead of use, so its cross-engine latency
            never gates the PE."""
            raw, sq = rawsq
            nrm2 = tiny_pool.tile([128, SUB, 2], F32, tag="nrm2")
            for hh in range(2):
                sl = slice(hh * HS, hh * HS + HS)
                nc.vector.tensor_reduce(nrm2[:, sl], sq[:, sl], axis=mybir.AxisListType.X, op=AluOpType.add)
            nrm2f = nrm2.rearrange("a p c -> a (p c)")
            nc.vector.tensor_scalar(out=nrm2f, in0=nrm2f, scalar1=1.0 / 128.0,
                                    scalar2=0.0, op0=AluOpType.mult, op1=AluOpType.add)

            # rinv = 1/sqrt(nrm2*128) via Newton on x' = nrm2 ~ 1
            yv = tiny_pool.tile([128, SUB, 2], F32, tag="newty")
            tv = tiny_pool.tile([128, SUB, 2], F32, tag="newtt")
            yvf = yv.rearrange("a p c -> a (p c)")
            tvf = tv.rearrange("a p c -> a (p c)")
            nc.vector.tensor_scalar(out=yvf, in0=nrm2f, scalar1=-0.5, scalar2=1.5,
                                    op0=AluOpType.mult, op1=AluOpType.add)
            for _ in range(3):
                nc.vector.tensor_mul(tvf, yvf, yvf)
                nc.vector.tensor_mul(tvf, tvf, nrm2f)
                nc.vector.tensor_scalar(out=tvf, in0=tvf, scalar1=-0.5, scalar2=1.5,
                                        op0=AluOpType.mult, op1=AluOpType.add)
                nc.vector.tensor_mul(yvf, yvf, tvf)
            nc.vector.tensor_scalar(out=yvf, in0=yvf, scalar1=float(1.0 / np.sqrt(128.0)),
                                    scalar2=0.0, op0=AluOpType.mult, op1=AluOpType.add)

            ctxn = norm_pool.tile([128, SUB, 128], F32, tag="ctxn")
            entn = norm_pool.tile([128, SUB, 128], F32, tag="entn")
            ctxb = norm_pool.tile([128, SUB, 128], BF16, tag="ctxb")
            entb = norm_pool.tile([128, SUB, 128], BF16, tag="entb")
            for hh in range(2):
                sl = slice(hh * HS, hh * HS + HS)
                rinv_c = yv[:, sl, 0:1].broadcast_to([128, HS, 128])
                rinv_e = yv[:, sl, 1:2].broadcast_to([128, HS, 128])
                nc.gpsimd.tensor_tensor(out=ctxn[:, sl], in0=raw[:, sl, 0, :], in1=rinv_c, op=AluOpType.mult)
                nc.gpsimd.tensor_tensor(out=entn[:, sl], in0=raw[:, sl, 1, :], in1=rinv_e, op=AluOpType.mult)
                nc.gpsimd.tensor_copy(entb[:, sl], entn[:, sl])
            return ctxn, entn, ctxb, entb

        def norm_casts(st):
            """bf16 copy of ctx_n (chunk0 source for the xbar transpose) on
            DVE; emitted late, when normalize is done."""
            ctxn, entn, ctxb, entb = st
            nc.vector.tensor_copy(ctxb, ctxn)

        def sim_stage_a(st, q):
            """Stage A of one 4-pair group: fp32 transposes -> sim -> argmax
            one-hot; ctx chunk of the MLP input goes out via DMA xbar
            transpose of ctxb. Returns (x_sb, oh, pbase)."""
            ctxn, entn, ctxb, entb = st
            pbase = q * GRP
            cpts = []
            for h in range(2):
                tp = ps_tp.tile([128, 2, 2, 128], F32, tag="tp")
                for j in range(2):
                    p = pbase + 2 * h + j
                    nc.tensor.transpose(tp[:, 0, j, :], ctxn[:, p, :], ident)
                    nc.tensor.transpose(tp[:, 1, j, :], entn[:, p, :], ident)
                cpt = pre_pool.tile([128, 2, 2, 128], F32, tag="cpt")
                nc.vector.tensor_copy(cpt, tp)
                cpts.append(cpt)

            sim = ps_sim.tile([128, GRP, 128], F32, tag="sim")
            for j in range(GRP):
                h, jj = divmod(j, 2)
                nc.tensor.matmul(sim[:, j, :], lhsT=cpts[h][:, 0, jj, :],
                                 rhs=cpts[h][:, 1, jj, :])
            mx = tiny_pool.tile([128, GRP], F32, tag="mx")
            nc.vector.tensor_reduce(mx, sim, axis=mybir.AxisListType.X, op=AluOpType.max)
            oh = pre_pool.tile([128, GRP, 128], BF16, tag="oh")
            nc.vector.tensor_tensor(
                out=oh, in0=sim,
                in1=mx.unsqueeze(2).broadcast_to([128, GRP, 128]),
                op=AluOpType.is_equal,
            )
            x_sb = x_pool.tile([128, 2, GRP, 128], BF16, tag="x")
            # chunk0 = ctx_nT bf16 via DMA xbar transpose (no PE/DVE cost)
            for j in range(GRP):
                nc.sync.dma_start_transpose(x_sb[:, 0, j, :], ctxb[:, pbase + j, :])
            return x_sb, oh, pbase

        def sim_stage_b(st, ab):
            """Stage B (a full MLP layer later): one-hot xbar transpose ->
            gather -> gathered chunk cast. oh is long since written."""
            ctxn, entn, ctxb, entb = st
            x_sb, oh, pbase = ab
            ohT = pre_pool.tile([128, GRP, 128], BF16, tag="ohT")
            for j in range(GRP):
                nc.sync.dma_start_transpose(ohT[:, j, :], oh[:, j, :])
            gat = ps_scr.tile([128, GRP, 128], F32, tag="scr")
            for j in range(GRP):
                nc.tensor.matmul(gat[:, j, :], lhsT=entb[:, pbase + j, :], rhs=ohT[:, j, :])
            nc.vector.tensor_copy(x_sb[:, 1], gat)  # chunk1 bf16
            return x_sb

        def mlp_subgroup(s, x_tiles, emits):
            """MLP for all 16 pairs (2 supergroups). `emits` maps layer index
            -> list of callbacks run BETWEEN the mc=0 and mc=1 halves of that
            layer (PE gap-filler exactly where the mm-ring waits on ACT)."""
            xins = [
                [[x_tiles[2 * qq + g][:, kc].rearrange("a g d -> a (g d)") for kc in range(2)]
                 for g in range(2)]
                for qq in range(2)
            ]
            for i in range(N_HIDDEN):
                yas = []
                for qq in range(2):
                    ya = y_pool.tile([128, 2, 2, GRP * 128], BF16, tag="y")
                    yas.append(ya)
                for mc in range(2):
                    for qq in range(2):
                        mm = ps_mlp.tile([128, 2, GRP * 128], F32, tag="mm")
                        for g in range(2):
                            nc.tensor.matmul(mm[:, g, :], lhsT=wt_sb[:, i, 0, mc, :],
                                             rhs=xins[qq][g][0], start=True, stop=False)
                            nc.tensor.matmul(mm[:, g, :], lhsT=wt_sb[:, i, 1, mc, :],
                                             rhs=xins[qq][g][1], start=False, stop=True)
                        nc.scalar.activation(
                            out=yas[qq][:, mc].rearrange("a g d -> a (g d)"),
                            in_=mm.rearrange("a g d -> a (g d)"),
                            func=AF.Tanh,
                            bias=bias_sb[:, 2 * i + mc : 2 * i + mc + 1],
                        )
                    if mc == 0:
                        for cb in emits.get(i, ()):
                            cb()
                xins = [[[yas[qq][:, kc, g] for kc in range(2)] for g in range(2)]
                        for qq in range(2)]
            # ya_last = xins source for ybar
            return yas

        def emit_ybar(ya_last):
            """Sum y6 over l per (pair, mc) on DVE (bf16 reads, fp32 accum),
            then a bf16 copy for the tiny wout matmul."""
            ybars = []
            for qq in range(2):
                ybar = ybar_pool.tile([128, 2, 2, GRP], F32, tag="ybar")
                ybarb = ybar_pool.tile([128, 2, 2, GRP], BF16, tag="ybarb")
                ya_v = ya_last[qq].rearrange("a mc g (p l) -> a mc g p l", p=GRP)
                for mc in range(2):
                    nc.vector.tensor_reduce(ybar[:, mc], ya_v[:, mc],
                                            axis=mybir.AxisListType.X, op=AluOpType.add)
                nc.vector.tensor_copy(ybarb, ybar)
                ybars.append(ybarb)
            return ybars

        def emit_wout_for(s, ybars, res):
            """Tiny bf16 wout matmuls for subgroup s + copy to res. Emitted
            mid-layer-1 of the next subgroup, so ybar is long done."""
            for qq in range(2):
                wo = ps_mlp.tile([1, 2, GRP], F32, tag="mm")
                for mc in range(2):
                    nc.tensor.matmul(wo.rearrange("a t g -> a (t g)"),
                                     lhsT=wout_sb[:, mc : mc + 1],
                                     rhs=ybars[qq][:, mc].rearrange("a g p -> a (g p)"),
                                     start=(mc == 0), stop=(mc == 1))
                col = (s % n_blk) * SUB + qq * 2 * GRP
                nc.vector.tensor_copy(res[0:1, col : col + 2 * GRP],
                                      wo.rearrange("a t g -> a (t g)"))

        def finalize_res(res, blk):
            g0 = blk * UNROLL
            # res += L * b_out  (sum over L rows of constant bias)
            nc.vector.tensor_scalar(out=res, in0=res, scalar1=bout128[0:1, 0:1],
                                    scalar2=0.0, op0=AluOpType.add, op1=AluOpType.add)
            nc.sync.dma_start(out=out_dram[0:1, g0 : g0 + UNROLL], in_=res)

        # ---------- software pipeline ----------
        rawsq = {0: dma_sq_stage(0), 1: dma_sq_stage(1)}
        sts = {0: norm_early(rawsq[0])}
        norm_casts(sts[0])
        sts[1] = norm_early(rawsq[1])
        norm_casts(sts[1])
        rawsq[2] = dma_sq_stage(2)

        # prologue: sim stages for subgroup 0, serial
        x_cur = []
        for q in range(SUB // GRP):
            ab = sim_stage_a(sts[0], q)
            x_cur.append(sim_stage_b(sts[0], ab))

        res = None
        prev_wout = None        # (s_prev, ybars_prev, res_prev)
        for s in range(n_sub_total):
            if s % n_blk == 0:
                res = res_pool.tile([1, UNROLL], F32, tag="res")
            if s + 2 < n_sub_total:
                sts[s + 2] = norm_early(rawsq[s + 2])
            if s + 3 < n_sub_total:
                rawsq[s + 3] = dma_sq_stage(s + 3)

            st_next = sts.get(s + 1)
            st_next2 = sts.get(s + 2)
            x_next = []
            ab_pend = []

            emits = {}
            if prev_wout is not None:
                ps_, pybars_, pres_ = prev_wout

                def emit_wout(_s=ps_, _y=pybars_, _r=pres_):
                    emit_wout_for(_s, _y, _r)
                    if _s % n_blk == n_blk - 1:
                        finalize_res(_r, _s // n_blk)

                emits.setdefault(1, []).append(emit_wout)
            if st_next is not None:
                for k in range(4):
                    def emit_a(_k=k, _st=st_next):
                        ab_pend.append(sim_stage_a(_st, _k))

                    def emit_b(_k=k, _st=st_next):
                        x_next.append(sim_stage_b(_st, ab_pend[_k]))

                    emits.setdefault(1 + k, []).append(emit_a)
                    emits.setdefault(2 + k, []).append(emit_b)
            if st_next2 is not None:
                def emit_casts(_st=st_next2):
                    norm_casts(_st)

                emits.setdefault(5, []).append(emit_casts)

            ya_last = mlp_subgroup(s, x_cur, emits)
            ybars = emit_ybar(ya_last)
            prev_wout = (s, ybars, res)
            x_cur = x_next

        # drain the last subgroup's wout + final block
        ps_, pybars_, pres_ = prev_wout
        emit_wout_for(ps_, pybars_, pres_)
        finalize_res(pres_, ps_ // n_blk)

    nc.compile()
    return nc


def _prep_weights(Ws, bs, W_out, b_out):
    Ws = np.asarray(Ws, dtype=np.float32)
    bs = np.asarray(bs, dtype=np.float32)
    W_out = np.asarray(W_out, dtype=np.float32)
    b_out = np.asarray(b_out, dtype=np.float32)
    # wt[a, i, kc, mc, b] = Ws[i, mc*128+b, kc*128+a]
    wt = np.transpose(
        Ws.reshape(N_HIDDEN, 2, 128, 2, 128),  # [i, mc, b, kc, a]
        (4, 0, 3, 1, 2),
    ).reshape(128, N_HIDDEN * 2 * 2 * 128)
    wt = np.ascontiguousarray(wt.astype(ml_dtypes.bfloat16))
    wout = np.ascontiguousarray(W_out.reshape(2, 128).T.astype(ml_dtypes.bfloat16))
    bias = np.ascontiguousarray(
        np.transpose(bs.reshape(N_HIDDEN, 2, 128), (2, 0, 1)).reshape(128, N_HIDDEN * 2)
    ).astype(np.float32)
    bout = b_out.reshape(1, 1).astype(np.float32)
    return wt, wout, bias, bout


def make_in_maps(context, Ws, bs, W_out, b_out):
    context = np.ascontiguousarray(np.asarray(context, dtype=np.float32))
    wt, wout, bias, bout = _prep_weights(Ws, bs, W_out, b_out)
    shards = context.reshape(N_CORES, PAIRS, 2, L, D)
    return [
        {"ctxpairs": np.ascontiguousarray(shards[i]), "wt": wt, "wout": wout,
         "bias": bias, "bout": bout}
        for i in range(N_CORES)
    ]


def kernel(context, Ws, bs, W_out, b_out):
    in_maps = make_in_maps(context, Ws, bs, W_out, b_out)
    if "nc" not in _cache:
        _cache["nc"] = _build_bass()
    nc = _cache["nc"]
    r = run_bass_kernel_spmd(nc, in_maps, core_ids=list(range(N_CORES)))
    out = np.concatenate([r.results[i]["out"].reshape(B // N_CORES, K) for i in range(N_CORES)], axis=0)
    return out.astype(np.float32)


if __name__ == "__main__":
    import reference
    inputs = reference.setup_inputs()
    inputs = {k: np.asarray(v) for k, v in inputs.items()}
    expected = np.asarray(reference.reference(**inputs))
    actual = kernel(**inputs)
    err = np.linalg.norm(actual - expected) / np.linalg.norm(expected)
    print("Relative error:", err)


# revision 9
# speedup vs baseline: 1.5903x; 1.5903x over previous
"""Trainium2 Bass kernel for MLP-with-SOM-cosine-similarity (retrieval_knn).

Reference computation per (b, k) pair:
  ctx, ent: [L=128, D=128] slices of context[b, k, 0/1]
  sim[l, m] = cos(ctx[l], ent[m]); idx[l] = argmax_m sim[l, m]
  x = [ctx_n | ent_n[idx]] -> 6x tanh(Linear(256,256)) -> dot W_out -> sum over l
Output: [B=64, K=64] f32.

Strategy: data-parallel over batch dim (8 cores x 8 batches = 512 pairs/core).

Engine budget (from trace analysis; per-core totals at 32 subgroups):
  - ACT (Scalar): tanh only, 24x [128,1024] instrs/subgroup = ~855us. Hard
    floor; nothing else may ride on ACT.
  - DVE: Newton rsqrt, ctxb/entb casts, PSUM->SBUF moves (cpt, gat), argmax
    (max reduce + is_equal), y6 row-sums. Target < ACT.
  - GpSimd: squares, norm reduce, normalize muls.
  - PE: fp32 transposes + fp32 sim (precision-mandatory: bf16 sim flips
    argmax -> 4.5e-2 rel err vs 2e-2 tol), bf16 MLP. Real streaming ~650us.
  - DMA xbar: one-hot transpose and ctx_nT bf16 chunk (16-bit only), freeing
    PE instrs and the DVE PSUM->SBUF copy they needed.
Pipeline (subgroup w): DMA+squares at w-3 | norms at w-2 | casts late at w-2
  | sim stage A (transposes+sim+argmax) mid-layers 1-4 of mlp(w-1), stage B
  (gather) mid-layers 2-5 | mlp at w | y6 row-sum at end of w | tiny wout
  matmuls mid-layer-1 of w+1. The mid-layer emission points keep the PE busy
  where it used to stall on the mm-ring waiting for ACT.
PSUM banks: tp(2) + sim(1) + scr/gat(1) + mlp(4, also hosts tiny wo) = 8.
"""

from contextlib import ExitStack

import numpy as np
import ml_dtypes

import concourse.bass as bass
import concourse.bacc as bacc
import concourse.tile as tile
from concourse import mybir
from concourse.alu_op_type import AluOpType
from concourse.bass_utils import run_bass_kernel_spmd
from concourse.masks import make_identity

BF16 = mybir.dt.bfloat16
F32 = mybir.dt.float32
AF = mybir.ActivationFunctionType

B, K, L, D = 64, 64, 128, 128
N_CORES = 8
PAIRS = (B // N_CORES) * K          # 512 pairs per core
N_HIDDEN = 6
SUB = 16                            # pairs per DMA subgroup
GRP = 4                             # pairs per PSUM group
UNROLL = 128                        # pairs per outer block

_cache = {}


def _build_bass():
    nc = bacc.Bacc("TRN2")

    ctx_dram = nc.dram_tensor("ctxpairs", [PAIRS, 2, L, D], F32, kind="ExternalInput")
    wt_dram = nc.dram_tensor("wt", [128, N_HIDDEN * 2 * 2 * 128], BF16, kind="ExternalInput")
    wout_dram = nc.dram_tensor("wout", [128, 2], BF16, kind="ExternalInput")
    bias_dram = nc.dram_tensor("bias", [128, N_HIDDEN * 2], F32, kind="ExternalInput")
    bout_dram = nc.dram_tensor("bout", [1, 1], F32, kind="ExternalInput")
    out_dram = nc.dram_tensor("out", [1, PAIRS], F32, kind="ExternalOutput")

    with ExitStack() as ctx:
        tc = ctx.enter_context(tile.TileContext(nc))
        const = ctx.enter_context(tc.tile_pool(name="const", bufs=1))
        raw_pool = ctx.enter_context(tc.tile_pool(name="raw", bufs=3))
        sq_pool = ctx.enter_context(tc.tile_pool(name="sq", bufs=2))
        norm_pool = ctx.enter_context(tc.tile_pool(name="norm", bufs=2))
        tiny_pool = ctx.enter_context(tc.tile_pool(name="tiny", bufs=4))
        pre_pool = ctx.enter_context(tc.tile_pool(name="pre", bufs=4))
        x_pool = ctx.enter_context(tc.tile_pool(name="xsb", bufs=4))
        y_pool = ctx.enter_context(tc.tile_pool(name="ysb", bufs=4))
        ybar_pool = ctx.enter_context(tc.tile_pool(name="ybar", bufs=4))
        res_pool = ctx.enter_context(tc.tile_pool(name="res", bufs=2))
        # PSUM: 8 banks total = tp(2) + sim(1) + scr(1) + mlp(4, also hosts wo)
        ps_tp = ctx.enter_context(tc.tile_pool(name="pstp", bufs=2, space="PSUM"))
        ps_sim = ctx.enter_context(tc.tile_pool(name="pssim", bufs=1, space="PSUM"))
        ps_scr = ctx.enter_context(tc.tile_pool(name="psscr", bufs=1, space="PSUM"))
        ps_mlp = ctx.enter_context(tc.tile_pool(name="psmlp", bufs=2, space="PSUM"))

        wt_sb = const.tile([128, N_HIDDEN, 2, 2, 128], BF16)
        nc.sync.dma_start(out=wt_sb, in_=wt_dram.rearrange("a (i kc mc b) -> a i kc mc b", i=N_HIDDEN, kc=2, mc=2))
        wout_sb = const.tile([128, 2], BF16)
        nc.sync.dma_start(out=wout_sb, in_=wout_dram[:, :])
        bias_sb = const.tile([128, N_HIDDEN * 2], F32)
        nc.sync.dma_start(out=bias_sb, in_=bias_dram[:, :])
        bout_sb = const.tile([1, 1], F32)
        nc.sync.dma_start(out=bout_sb, in_=bout_dram[:, :])
        ident = const.tile([128, 128], F32)
        make_identity(nc, ident)
        bout128 = const.tile([1, 1], F32)
        nc.vector.tensor_scalar(out=bout128, in0=bout_sb, scalar1=float(L), scalar2=0.0,
                                op0=AluOpType.mult, op1=AluOpType.add)

        n_blk = UNROLL // SUB           # subgroups per output block
        n_sub_total = PAIRS // SUB
        HS = SUB // 2

        def dma_sq_stage(s):
            """DMA subgroup s + squares on gpsimd. Emitted 3 subgroups ahead."""
            raw = raw_pool.tile([128, SUB, 2, 128], F32, tag="raw")
            nc.sync.dma_start(
                out=raw,
                in_=ctx_dram[s * SUB : s * SUB + SUB].rearrange("p c l d -> l p c d"),
            )
            sq = sq_pool.tile([128, SUB, 2, 128], F32, tag="sq")
            for hh in range(2):
                nc.gpsimd.tensor_mul(sq[:, hh * HS : hh * HS + HS],
                                     raw[:, hh * HS : hh * HS + HS],
                                     raw[:, hh * HS : hh * HS + HS])
            return raw, sq

        def norm_early(rawsq):
            """Norm reduce (gpsimd) + Newton rsqrt (DVE) + normalize (gpsimd).
            Emitted 2 subgroups ahead of use, so its cross-engine latency
            never gates the PE."""
            raw, sq = rawsq
            nrm2 = tiny_pool.tile([128, SUB, 2], F32, tag="nrm2")
            for hh in range(2):
                sl = slice(hh * HS, hh * HS + HS)
                nc.vector.tensor_reduce(nrm2[:, sl], sq[:, sl], axis=mybir.AxisListType.X, op=AluOpType.add)
            nrm2f = nrm2.rearrange("a p c -> a (p c)")
            nc.vector.tensor_scalar(out=nrm2f, in0=nrm2f, scalar1=1.0 / 128.0,
                                    scalar2=0.0, op0=AluOpType.mult, op1=AluOpType.add)

            # rinv = 1/sqrt(nrm2*128) via Newton on x' = nrm2 ~ 1
            yv = tiny_pool.tile([128, SUB, 2], F32, tag="newty")
            tv = tiny_pool.tile([128, SUB, 2], F32, tag="newtt")
            yvf = yv.rearrange("a p c -> a (p c)")
            tvf = tv.rearrange("a p c -> a (p c)")
            nc.vector.tensor_scalar(out=yvf, in0=nrm2f, scalar1=-0.5, scalar2=1.5,
                                    op0=AluOpType.mult, op1=AluOpType.add)
            for _ in range(3):
                nc.vector.tensor_mul(tvf, yvf, yvf)
                nc.vector.tensor_mul(tvf, tvf, nrm2f)
                nc.vector.tensor_scalar(out=tvf, in0=tvf, scalar1=-0.5, scalar2=1.5,
                                        op0=AluOpType.mult, op1=AluOpType.add)
                nc.vector.tensor_mul(yvf, yvf, tvf)
            nc.vector.tensor_scalar(out=yvf, in0=yvf, scalar1=float(1.0 / np.sqrt(128.0)),
                                    scalar2=0.0, op0=AluOpType.mult, op1=AluOpType.add)

            ctxn = norm_pool.tile([128, SUB, 128], F32, tag="ctxn")
            entn = norm_pool.tile([128, SUB, 128], F32, tag="entn")
            ctxb = norm_pool.tile([128, SUB, 128], BF16, tag="ctxb")
            entb = norm_pool.tile([128, SUB, 128], BF16, tag="entb")
            for hh in range(2):
                sl = slice(hh * HS, hh * HS + HS)
                rinv_c = yv[:, sl, 0:1].broadcast_to([128, HS, 128])
                rinv_e = yv[:, sl, 1:2].broadcast_to([128, HS, 128])
                nc.gpsimd.tensor_tensor(out=ctxn[:, sl], in0=raw[:, sl, 0, :], in1=rinv_c, op=AluOpType.mult)
                nc.gpsimd.tensor_tensor(out=entn[:, sl], in0=raw[:, sl, 1, :], in1=rinv_e, op=AluOpType.mult)
                nc.gpsimd.tensor_copy(entb[:, sl], entn[:, sl])
            return ctxn, entn, ctxb, entb

        def norm_casts(st):
            """bf16 copy of ctx_n (chunk0 source for the xbar transpose) on
            DVE; emitted late, when normalize is done."""
            ctxn, entn, ctxb, entb = st
            nc.vector.tensor_copy(ctxb, ctxn)

        def sim_stage_a(st, q):
            """Stage A of one 4-pair group: fp32 transposes -> sim -> argmax
            one-hot; ctx chunk of the MLP input goes out via DMA xbar
            transpose of ctxb. Returns (x_sb, oh, pbase)."""
            ctxn, entn, ctxb, entb = st
            pbase = q * GRP
            cpts = []
            for h in range(2):
                tp = ps_tp.tile([128, 2, 2, 128], F32, tag="tp")
                for j in range(2):
                    p = pbase + 2 * h + j
                    nc.tensor.transpose(tp[:, 0, j, :], ctxn[:, p, :], ident)
                    nc.tensor.transpose(tp[:, 1, j, :], entn[:, p, :], ident)
                cpt = pre_pool.tile([128, 2, 2, 128], F32, tag="cpt")
                nc.vector.tensor_copy(cpt, tp)
                cpts.append(cpt)

            sim = ps_sim.tile([128, GRP, 128], F32, tag="sim")
            for j in range(GRP):
                h, jj = divmod(j, 2)
                nc.tensor.matmul(sim[:, j, :], lhsT=cpts[h][:, 0, jj, :],
                                 rhs=cpts[h][:, 1, jj, :])
            mx = tiny_pool.tile([128, GRP], F32, tag="mx")
            nc.vector.tensor_reduce(mx, sim, axis=mybir.AxisListType.X, op=AluOpType.max)
            oh = pre_pool.tile([128, GRP, 128], BF16, tag="oh")
            nc.vector.tensor_tensor(
                out=oh, in0=sim,
                in1=mx.unsqueeze(2).broadcast_to([128, GRP, 128]),
                op=AluOpType.is_equal,
            )
            x_sb = x_pool.tile([128, 2, GRP, 128], BF16, tag="x")
            # chunk0 = ctx_nT bf16 via ONE DMA xbar transpose for all 4 pairs:
            # column blocks of the 2D input land as partition segments, i.e.
            # out[:, j, :] = T(in[:, j*128:(j+1)*128]).
            nc.sync.dma_start_transpose(
                x_sb[:, 0, :, :],
                ctxb[:, pbase : pbase + GRP, :].rearrange("a p d -> a (p d)"),
            )
            return x_sb, oh, pbase

        def sim_stage_b(st, ab):
            """Stage B (a full MLP layer later): one-hot xbar transpose ->
            gather -> gathered chunk cast. oh is long since written."""
            ctxn, entn, ctxb, entb = st
            x_sb, oh, pbase = ab
            ohT = pre_pool.tile([128, GRP, 128], BF16, tag="ohT")
            nc.sync.dma_start_transpose(ohT, oh.rearrange("a p d -> a (p d)"))
            gat = ps_scr.tile([128, GRP, 128], F32, tag="scr")
            for j in range(GRP):
                nc.tensor.matmul(gat[:, j, :], lhsT=entb[:, pbase + j, :], rhs=ohT[:, j, :])
            nc.vector.tensor_copy(x_sb[:, 1], gat)  # chunk1 bf16
            return x_sb

        def mlp_subgroup(s, x_tiles, emits):
            """MLP for all 16 pairs (2 supergroups). `emits` maps layer index
            -> list of callbacks run BETWEEN the mc=0 and mc=1 halves of that
            layer (PE gap-filler exactly where the mm-ring waits on ACT)."""
            xins = [
                [[x_tiles[2 * qq + g][:, kc].rearrange("a g d -> a (g d)") for kc in range(2)]
                 for g in range(2)]
                for qq in range(2)
            ]
            for i in range(N_HIDDEN):
                yas = []
                for qq in range(2):
                    ya = y_pool.tile([128, 2, 2, GRP * 128], BF16, tag="y")
                    yas.append(ya)
                for mc in range(2):
                    for qq in range(2):
                        mm = ps_mlp.tile([128, 2, GRP * 128], F32, tag="mm")
                        for g in range(2):
                            nc.tensor.matmul(mm[:, g, :], lhsT=wt_sb[:, i, 0, mc, :],
                                             rhs=xins[qq][g][0], start=True, stop=False)
                            nc.tensor.matmul(mm[:, g, :], lhsT=wt_sb[:, i, 1, mc, :],
                                             rhs=xins[qq][g][1], start=False, stop=True)
                        nc.scalar.activation(
                            out=yas[qq][:, mc].rearrange("a g d -> a (g d)"),
                            in_=mm.rearrange("a g d -> a (g d)"),
                            func=AF.Tanh,
                            bias=bias_sb[:, 2 * i + mc : 2 * i + mc + 1],
                        )
                    if mc == 0:
                        for cb in emits.get(i, ()):
                            cb()
                xins = [[[yas[qq][:, kc, g] for kc in range(2)] for g in range(2)]
                        for qq in range(2)]
            # ya_last = xins source for ybar
            return yas

        def emit_ybar(ya_last):
            """Sum y6 over l per (pair, mc) on DVE (bf16 reads, fp32 accum),
            then a bf16 copy for the tiny wout matmul."""
            ybars = []
            for qq in range(2):
                ybar = ybar_pool.tile([128, 2, 2, GRP], F32, tag="ybar")
                ybarb = ybar_pool.tile([128, 2, 2, GRP], BF16, tag="ybarb")
                ya_v = ya_last[qq].rearrange("a mc g (p l) -> a mc g p l", p=GRP)
                for mc in range(2):
                    nc.vector.tensor_reduce(ybar[:, mc], ya_v[:, mc],
                                            axis=mybir.AxisListType.X, op=AluOpType.add)
                nc.vector.tensor_copy(ybarb, ybar)
                ybars.append(ybarb)
            return ybars

        def emit_wout_for(s, ybars, res):
            """Tiny bf16 wout matmuls for subgroup s + copy to res. Emitted
            mid-layer-1 of the next subgroup, so ybar is long done."""
            for qq in range(2):
                wo = ps_mlp.tile([1, 2, GRP], F32, tag="mm")
                for mc in range(2):
                    nc.tensor.matmul(wo.rearrange("a t g -> a (t g)"),
                                     lhsT=wout_sb[:, mc : mc + 1],
                                     rhs=ybars[qq][:, mc].rearrange("a g p -> a (g p)"),
                                     start=(mc == 0), stop=(mc == 1))
                col = (s % n_blk) * SUB + qq * 2 * GRP
                nc.vector.tensor_copy(res[0:1, col : col + 2 * GRP],
                                      wo.rearrange("a t g -> a (t g)"))

        def finalize_res(res, blk):
            g0 = blk * UNROLL
            # res += L * b_out  (sum over L rows of constant bias)
            nc.vector.tensor_scalar(out=res, in0=res, scalar1=bout128[0:1, 0:1],
                                    scalar2=0.0, op0=AluOpType.add, op1=AluOpType.add)
            nc.sync.dma_start(out=out_dram[0:1, g0 : g0 + UNROLL], in_=res)

        # ---------- software pipeline ----------
        rawsq = {0: dma_sq_stage(0), 1: dma_sq_stage(1)}
        sts = {0: norm_early(rawsq[0])}
        norm_casts(sts[0])
        sts[1] = norm_early(rawsq[1])
        norm_casts(sts[1])
        rawsq[2] = dma_sq_stage(2)

        # prologue: sim stages for subgroup 0, serial
        x_cur = []
        for q in range(SUB // GRP):
            ab = sim_stage_a(sts[0], q)
            x_cur.append(sim_stage_b(sts[0], ab))

        res = None
        prev_wout = None        # (s_prev, ybars_prev, res_prev)
        for s in range(n_sub_total):
            if s % n_blk == 0:
                res = res_pool.tile([1, UNROLL], F32, tag="res")
            if s + 2 < n_sub_total:
                sts[s + 2] = norm_early(rawsq[s + 2])
            if s + 3 < n_sub_total:
                rawsq[s + 3] = dma_sq_stage(s + 3)

            st_next = sts.get(s + 1)
            st_next2 = sts.get(s + 2)
            x_next = []
            ab_pend = []

            emits = {}
            if prev_wout is not None:
                ps_, pybars_, pres_ = prev_wout

                def emit_wout(_s=ps_, _y=pybars_, _r=pres_):
                    emit_wout_for(_s, _y, _r)
                    if _s % n_blk == n_blk - 1:
                        finalize_res(_r, _s // n_blk)

                emits.setdefault(1, []).append(emit_wout)
            if st_next is not None:
                for k in range(4):
                    def emit_a(_k=k, _st=st_next):
                        ab_pend.append(sim_stage_a(_st, _k))

                    def emit_b(_k=k, _st=st_next):
                        x_next.append(sim_stage_b(_st, ab_pend[_k]))

                    emits.setdefault(1 + k, []).append(emit_a)
                    emits.setdefault(2 + k, []).append(emit_b)
            if st_next2 is not None:
                def emit_casts(_st=st_next2):
                    norm_casts(_st)

                emits.setdefault(5, []).append(emit_casts)

            ya_last = mlp_subgroup(s, x_cur, emits)
            ybars = emit_ybar(ya_last)
            prev_wout = (s, ybars, res)
            x_cur = x_next

        # drain the last subgroup's wout + final block
        ps_, pybars_, pres_ = prev_wout
        emit_wout_for(ps_, pybars_, pres_)
        finalize_res(pres_, ps_ // n_blk)

    nc.compile()
    return nc


def _prep_weights(Ws, bs, W_out, b_out):
    Ws = np.asarray(Ws, dtype=np.float32)
    bs = np.asarray(bs, dtype=np.float32)
    W_out = np.asarray(W_out, dtype=np.float32)
    b_out = np.asarray(b_out, dtype=np.float32)
    # wt[a, i, kc, mc, b] = Ws[i, mc*128+b, kc*128+a]
    wt = np.transpose(
        Ws.reshape(N_HIDDEN, 2, 128, 2, 128),  # [i, mc, b, kc, a]
        (4, 0, 3, 1, 2),
    ).reshape(128, N_HIDDEN * 2 * 2 * 128)
    wt = np.ascontiguousarray(wt.astype(ml_dtypes.bfloat16))
    wout = np.ascontiguousarray(W_out.reshape(2, 128).T.astype(ml_dtypes.bfloat16))
    bias = np.ascontiguousarray(
        np.transpose(bs.reshape(N_HIDDEN, 2, 128), (2, 0, 1)).reshape(128, N_HIDDEN * 2)
    ).astype(np.float32)
    bout = b_out.reshape(1, 1).astype(np.float32)
    return wt, wout, bias, bout


def make_in_maps(context, Ws, bs, W_out, b_out):
    context = np.ascontiguousarray(np.asarray(context, dtype=np.float32))
    wt, wout, bias, bout = _prep_weights(Ws, bs, W_out, b_out)
    shards = context.reshape(N_CORES, PAIRS, 2, L, D)
    return [
        {"ctxpairs": np.ascontiguousarray(shards[i]), "wt": wt, "wout": wout,
         "bias": bias, "bout": bout}
        for i in range(N_CORES)
    ]


def kernel(context, Ws, bs, W_out, b_out):
    in_maps = make_in_maps(context, Ws, bs, W_out, b_out)
    if "nc" not in _cache:
        _cache["nc"] = _build_bass()
    nc = _cache["nc"]
    r = run_bass_kernel_spmd(nc, in_maps, core_ids=list(range(N_CORES)))
    out = np.concatenate([r.results[i]["out"].reshape(B // N_CORES, K) for i in range(N_CORES)], axis=0)
    return out.astype(np.float32)


if __name__ == "__main__":
    import reference
    inputs = reference.setup_inputs()
    inputs = {k: np.asarray(v) for k, v in inputs.items()}
    expected = np.asarray(reference.reference(**inputs))
    actual = kernel(**inputs)
    err = np.linalg.norm(actual - expected) / np.linalg.norm(expected)
    print("Relative error:", err)
